# revision 34
# baseline (speedup 1.0000x reference)
"""Trainium2 Bass kernel for nn_MultiHeadAttention_34651796144477.

Head-parallel sharding: core h computes the full (normalized) attention
output for head h over all 4 batches; the host applies the Wo-weighted
combine across the 8 per-head partials (the "all-reduce over heads").

Low-rank restructure (V = x @ Wvd @ Wvu has rank D=64):
  QT[d,s], KT[d,s]  : projections, d on partitions
  Vd = x @ Wvd      : [t, 64] tiles (t on partitions)
  scoreT[t,s] = K Q^T, causal-masked in PSUM, exp via ACT (scale=1/8,
                bias=-EXPB; no max-subtraction — scores are bounded)
  odT[d', s] = sum_t [Vd | 1][t, d'] exp[t, s]   (d'=65: row 64 is the
                softmax denominator l[s], accumulated in the same matmul)
  out[s, e] = (odT[0:64, s].T @ Wvu) * (e^? / l[s])  — per-partition scale

All matmul operands are fp16 (PSUM fp32); upper-triangle tiles are skipped
and diagonal-region tiles are masked with -1e6 before exp.
"""

import os
import sys

try:
    import concourse  # noqa: F401  (resolves via the axon site paths)
except ImportError:
    for _p in ("/opt/trn_rl_repo",):
        if _p not in sys.path and os.path.isdir(_p):
            sys.path.append(_p)

import numpy as np

import concourse.bacc as bacc
import concourse.mybir as mybir
from concourse.tile import TileContext
from concourse.bass_utils import run_bass_kernel_spmd

B, S, E, D, H = 4, 2048, 512, 64, 8
P = 128                 # partition tile
NT = S // P             # 16 key tiles
NBLK = 4                # s blocks of 512 (4 row tiles each)
NE = E // P             # 4 e-chunks
EXPB = 3.0              # constant bias inside exp(): keeps fp16 attn tiles
                        # and the odT accumulator in range; cancels in out/l
F32 = mybir.dt.float32
F16 = mybir.dt.float16

LAST_RESULT = None      # test harness reads exec_time_ns off this

_prog_cache = {}


def _build_program():
    nc = bacc.Bacc(None, target_bir_lowering=False)

    xT = nc.declare_dram_parameter("xT", [B, E, S], F16, isOutput=False)
    wq = nc.declare_dram_parameter("wq", [E, D], F16, isOutput=False)
    wk = nc.declare_dram_parameter("wk", [E, D], F16, isOutput=False)
    wvd = nc.declare_dram_parameter("wvd", [E, D], F16, isOutput=False)
    wvu = nc.declare_dram_parameter("wvu", [D, E], F16, isOutput=False)
    # 4 mask variants [128, 512]: variant k masks columns < 128k fully and
    # the [128k, 128k+128) sub-tile triangularly (keep t<=s)
    maskd = nc.declare_dram_parameter("maskd", [P, P], F32, isOutput=False)
    ones_in = nc.declare_dram_parameter("ones_in", [P, 1], F16, isOutput=False)
    negb_in = nc.declare_dram_parameter("negb_in", [P, 1], F32, isOutput=False)
    out = nc.declare_dram_parameter("out", [B, S, E], F32, isOutput=True)

    Exp = mybir.ActivationFunctionType.Exp

    with TileContext(nc) as tc:
        with tc.tile_pool(name="const", bufs=1) as cpool, \
             tc.tile_pool(name="xt", bufs=8) as xtpool, \
             tc.tile_pool(name="qk", bufs=2) as qkpool, \
             tc.tile_pool(name="vd", bufs=9) as vdpool, \
             tc.tile_pool(name="et", bufs=20) as epool, \
             tc.tile_pool(name="od", bufs=3) as odpool, \
             tc.tile_pool(name="ob", bufs=4) as opool, \
             tc.tile_pool(name="rc", bufs=4) as rcpool, \
             tc.tile_pool(name="ps", bufs=5, space="PSUM") as pspool, \
             tc.tile_pool(name="pod", bufs=2, space="PSUM") as podpool, \
             tc.tile_pool(name="pf", bufs=1, space="PSUM") as pfpool:

            # ---- constants / weights ----
            wq_sb = cpool.tile([P, NE * D], F16)    # chunk c: e in [128c,128c+128)
            wk_sb = cpool.tile([P, NE * D], F16)
            wvd_sb = cpool.tile([P, NE * D], F16)
            for c in range(NE):
                nc.gpsimd.dma_start(out=wq_sb[:, D * c:D * (c + 1)],
                                    in_=wq[P * c:P * (c + 1), :])
                nc.gpsimd.dma_start(out=wk_sb[:, D * c:D * (c + 1)],
                                    in_=wk[P * c:P * (c + 1), :])
                nc.gpsimd.dma_start(out=wvd_sb[:, D * c:D * (c + 1)],
                                    in_=wvd[P * c:P * (c + 1), :])
            wvu_sb = cpool.tile([D, E], F16)
            nc.gpsimd.dma_start(out=wvu_sb[:], in_=wvu[:])
            mask_sb = cpool.tile([P, P], F32)
            nc.gpsimd.dma_start(out=mask_sb[:], in_=maskd[:])
            ones_sb = cpool.tile([P, 1], F16)
            nc.gpsimd.dma_start(out=ones_sb[:], in_=ones_in[:])
            ones4_sb = cpool.tile([P, 4], F16)
            for _c4 in range(4):
                nc.gpsimd.dma_start(out=ones4_sb[:, _c4:_c4 + 1], in_=ones_in[:])
            negb_sb = cpool.tile([P, 1], F32)
            nc.gpsimd.dma_start(out=negb_sb[:], in_=negb_in[:])

            def emit_xt(b):
                xt = [xtpool.tile([P, S], F16, tag="xt", name=f"xt{b}_{c}")
                      for c in range(NE)]
                # column-quarter DMAs: the sc=0 projection group only needs
                # the first quarter of each chunk (subtile deps unblock it)
                for q in range(4):
                    ql = slice(512 * q, 512 * (q + 1))
                    for c in range(NE):
                        nc.sync.dma_start(out=xt[c][:, ql],
                                          in_=xT[b, P * c:P * (c + 1), ql])
                return xt

            def proj_steps(b, xt):
                """Allocate qt/kt/vd tiles for batch b; return emission thunks
                (one per PSUM group) so projection work can be interleaved
                into the previous batch's attention blocks."""
                qt_sb = qkpool.tile([D, S], F16, tag="qt", name=f"qt{b}")
                kt_sb = qkpool.tile([D, S], F16, tag="kt", name=f"kt{b}")
                vdq = [vdpool.tile([P, 4 * (D + 1)], F16, tag="vd",
                                   name=f"vdq{b}_{q}") for q in range(4)]
                vd = [vdq[t // 4][:, (D + 1) * (t % 4):(D + 1) * (t % 4 + 1)]
                      for t in range(NT)]
                steps = []

                def qk_step(sc, wsb, dst, on_dve):
                    def f():
                        sl = slice(512 * sc, 512 * (sc + 1))
                        pq = pspool.tile([D, 512], F32, tag="mm",
                                         name=f"pj{b}_{sc}_{id(wsb)}")
                        for c in range(NE):
                            nc.tensor.matmul(pq[:], wsb[:, D * c:D * (c + 1)],
                                             xt[c][:, sl],
                                             start=(c == 0), stop=(c == NE - 1))
                        if on_dve:
                            nc.vector.tensor_copy(dst[:, sl], pq[:])
                        else:
                            nc.scalar.copy(dst[:, sl], pq[:])
                    return f

                def vd_step(q):
                    def f():
                        pv = pspool.tile([P, 4 * D], F32, tag="mm",
                                         name=f"pv{b}_{q}")
                        for i in range(4):
                            t = 4 * q + i
                            for c in range(NE):
                                nc.tensor.matmul(
                                    pv[:, D * i:D * (i + 1)],
                                    xt[c][:, P * t:P * (t + 1)],
                                    wvd_sb[:, D * c:D * (c + 1)],
                                    start=(c == 0), stop=(c == NE - 1),
                                    skip_group_check=True)
                        # one strided copy: pv [128,(i d)] -> vdq [128,(i d+1)]
                        nc.vector.tensor_copy(
                            vdq[q][:, :].rearrange("p (i d) -> p i d",
                                                   d=D + 1)[:, :, 0:D],
                            pv[:, :].rearrange("p (i d) -> p i d", d=D))
                        nc.vector.tensor_copy(
                            vdq[q][:, :].rearrange("p (i d) -> p i d",
                                                   d=D + 1)[:, :, D:D + 1],
                            ones4_sb[:, :].rearrange("p (i o) -> p i o", o=1))
                    return f

                for sc in range(NBLK):
                    steps.append(qk_step(sc, wq_sb, qt_sb, False))
                    steps.append(qk_step(sc, wk_sb, kt_sb, False))
                for q in range(4):
                    steps.append(vd_step(q))
                return qt_sb, kt_sb, vd, steps

            def attention_block(b, p, qt_sb, kt_sb, vd):
                jmax = 4 * p + 3
                pod = podpool.tile([D + 1, 512], F32, tag="od",
                                   name=f"pod{b}_{p}")
                for j in range(jmax + 1):
                    w0 = P * (j - 4 * p) if j >= 4 * p else 0
                    psc = pspool.tile([P, 512], F32, tag="mm",
                                      name=f"sc{b}_{p}_{j}")
                    et = epool.tile([P, 512], F16, tag="e",
                                    name=f"et{b}_{p}_{j}")
                    nc.tensor.matmul(
                        psc[:, w0:512],
                        kt_sb[:, P * j:P * (j + 1)],
                        qt_sb[:, 512 * p + w0:512 * (p + 1)],
                        start=True, stop=True)
                    if j >= 4 * p:
                        nc.vector.tensor_add(
                            psc[:, w0:w0 + P],
                            psc[:, w0:w0 + P], mask_sb[:])
                    nc.scalar.activation(et[:, w0:512], psc[:, w0:512],
                                         Exp, scale=0.125, bias=negb_sb[:])
                    nc.tensor.matmul(pod[:, w0:512], vd[j][:],
                                     et[:, w0:512],
                                     start=(j == 0), stop=(j == jmax),
                                     skip_group_check=True)

                od_sb = odpool.tile([D + 1, 512], F16, tag="ods",
                                    name=f"ods{b}_{p}")
                nc.vector.tensor_copy(od_sb[:], pod[:])
                lrow0 = rcpool.tile([1, 512], F16, tag="lr", name=f"lr{b}_{p}")
                nc.sync.dma_start(out=lrow0[:], in_=od_sb[D:D + 1, :])
                rrow = rcpool.tile([1, 512], F16, tag="rr", name=f"rr{b}_{p}")
                with nc.allow_low_precision(reason="1/l fp16 transpose trick"):
                    nc.vector.reciprocal(rrow[:], lrow0[:])
                prec = pfpool.tile([P, 4], F32, tag="f", name=f"prec{b}_{p}")
                for c in range(4):
                    nc.tensor.matmul(prec[:, c:c + 1],
                                     rrow[0:1, P * c:P * (c + 1)],
                                     ones_sb[0:1, :],
                                     start=True, stop=True)
                rec = rcpool.tile([P, 4], F32, tag="rc", name=f"rc{b}_{p}")
                nc.vector.tensor_copy(rec[:], prec[:])
                for k in range(4):
                    pf = pfpool.tile([P, E], F32, tag="f",
                                     name=f"pf{b}_{p}_{k}")
                    nc.tensor.matmul(pf[:], od_sb[0:D, P * k:P * (k + 1)],
                                     wvu_sb[:], start=True, stop=True)
                    osb = opool.tile([P, E], F32, tag="o",
                                     name=f"o{b}_{p}_{k}")
                    nc.vector.tensor_scalar_mul(osb[:], pf[:], rec[:, k:k + 1])
                    row = 512 * p + P * k
                    nc.sync.dma_start(out=out[b, row:row + P, :], in_=osb[:])

            # prologue: batch 0 projections up-front
            xt = emit_xt(0)
            qt_sb, kt_sb, vd, _steps = None, None, None, None
            qt_sb, kt_sb, vd, st = proj_steps(0, xt)
            for f in st:
                f()
            for b in range(B):
                nxt = None
                if b + 1 < B:
                    xt_n = emit_xt(b + 1)
                    nxt = proj_steps(b + 1, xt_n)
                for p in range(NBLK):
                    attention_block(b, p, qt_sb, kt_sb, vd)
                    if nxt is not None:
                        for f in nxt[3][6 * p:6 * (p + 1)]:
                            f()
                if nxt is not None:
                    qt_sb, kt_sb, vd = nxt[0], nxt[1], nxt[2]

    nc.compile()
    return nc


def _masks():
    """mask variant k, rows [128k,128k+128): cols < 128k -> -1e6,
    diagonal sub-tile triangular (keep t<=s), cols > diag -> 0."""
    tt, ss = np.meshgrid(np.arange(P), np.arange(P), indexing="ij")
    return np.where(tt <= ss, 0.0, -1.0e6).astype(np.float32)


def kernel(x, Wq, Wk, Wvd, Wvu, Wo):
    global LAST_RESULT
    if "nc" not in _prog_cache:
        _prog_cache["nc"] = _build_program()
    nc = _prog_cache["nc"]

    xT = np.ascontiguousarray(np.asarray(x).transpose(0, 2, 1)).astype(np.float16)
    maskd = _masks()
    ones_in = np.ones((P, 1), np.float16)

    in_maps = []
    for h in range(H):
        in_maps.append({
            "xT": xT,
            "wq": np.ascontiguousarray(Wq[h]).astype(np.float16),
            "wk": np.ascontiguousarray(Wk[h]).astype(np.float16),
            "wvd": np.ascontiguousarray(Wvd[h]).astype(np.float16),
            "wvu": np.ascontiguousarray(Wvu[h]).astype(np.float16),
            "maskd": maskd,
            "ones_in": ones_in,
            "negb_in": np.full((P, 1), -EXPB, np.float32),
        })

    res = run_bass_kernel_spmd(nc, in_maps, list(range(H)))
    LAST_RESULT = res

    out = np.zeros((B, S, E), np.float32)
    wo = np.asarray(Wo, np.float32).reshape(H)
    for h in range(H):
        out += wo[h] * res.results[h]["out"].astype(np.float32)
    return out


# revision 38
# speedup vs baseline: 1.0169x; 1.0169x over previous
"""Trainium2 Bass kernel for nn_MultiHeadAttention_34651796144477.

Head-parallel sharding: core h computes the full (normalized) attention
output for head h over all 4 batches; the host applies the Wo-weighted
combine across the 8 per-head partials (the "all-reduce over heads").

Low-rank restructure (V = x @ Wvd @ Wvu has rank D=64):
  QT[d,s], KT[d,s]  : projections, d on partitions
  Vd = x @ Wvd      : [t, 64] tiles (t on partitions)
  scoreT[t,s] = K Q^T, causal-masked in PSUM, exp via ACT (scale=1/8,
                bias=-EXPB; no max-subtraction — scores are bounded)
  odT[d', s] = sum_t [Vd | 1][t, d'] exp[t, s]   (d'=65: row 64 is the
                softmax denominator l[s], accumulated in the same matmul)
  out[s, e] = (odT[0:64, s].T @ Wvu) * (e^? / l[s])  — per-partition scale

All matmul operands are fp16 (PSUM fp32); upper-triangle tiles are skipped
and diagonal-region tiles are masked with -1e6 before exp.
"""

import os
import sys

try:
    import concourse  # noqa: F401  (resolves via the axon site paths)
except ImportError:
    for _p in ("/opt/trn_rl_repo",):
        if _p not in sys.path and os.path.isdir(_p):
            sys.path.append(_p)

import numpy as np

import concourse.bacc as bacc
import concourse.mybir as mybir
from concourse.tile import TileContext
from concourse.bass_utils import run_bass_kernel_spmd

B, S, E, D, H = 4, 2048, 512, 64, 8
P = 128                 # partition tile
NT = S // P             # 16 key tiles
NBLK = 4                # s blocks of 512 (4 row tiles each)
NE = E // P             # 4 e-chunks
EXPB = 3.0              # constant bias inside exp(): keeps fp16 attn tiles
                        # and the odT accumulator in range; cancels in out/l
F32 = mybir.dt.float32
F16 = mybir.dt.float16

LAST_RESULT = None      # test harness reads exec_time_ns off this

_prog_cache = {}


def _build_program():
    nc = bacc.Bacc(None, target_bir_lowering=False)

    xT = nc.declare_dram_parameter("xT", [B, E, S], F16, isOutput=False)
    wq = nc.declare_dram_parameter("wq", [E, D], F16, isOutput=False)
    wk = nc.declare_dram_parameter("wk", [E, D], F16, isOutput=False)
    wvd = nc.declare_dram_parameter("wvd", [E, D], F16, isOutput=False)
    wvu = nc.declare_dram_parameter("wvu", [D, E], F16, isOutput=False)
    # 4 mask variants [128, 512]: variant k masks columns < 128k fully and
    # the [128k, 128k+128) sub-tile triangularly (keep t<=s)
    maskd = nc.declare_dram_parameter("maskd", [P, P], F32, isOutput=False)
    ones_in = nc.declare_dram_parameter("ones_in", [P, 1], F16, isOutput=False)
    negb_in = nc.declare_dram_parameter("negb_in", [P, 1], F32, isOutput=False)
    out = nc.declare_dram_parameter("out", [B, S, E], F32, isOutput=True)

    Exp = mybir.ActivationFunctionType.Exp

    with TileContext(nc) as tc:
        with tc.tile_pool(name="const", bufs=1) as cpool, \
             tc.tile_pool(name="xt", bufs=8) as xtpool, \
             tc.tile_pool(name="qk", bufs=4) as qkpool, \
             tc.tile_pool(name="vd", bufs=9) as vdpool, \
             tc.tile_pool(name="et", bufs=20) as epool, \
             tc.tile_pool(name="od", bufs=4) as odpool, \
             tc.tile_pool(name="ob", bufs=8) as opool, \
             tc.tile_pool(name="rc", bufs=8) as rcpool, \
             tc.tile_pool(name="ps", bufs=5, space="PSUM") as pspool, \
             tc.tile_pool(name="pod", bufs=2, space="PSUM") as podpool, \
             tc.tile_pool(name="pf", bufs=1, space="PSUM") as pfpool:

            # ---- constants / weights ----
            wq_sb = cpool.tile([P, NE * D], F16)    # chunk c: e in [128c,128c+128)
            wk_sb = cpool.tile([P, NE * D], F16)
            wvd_sb = cpool.tile([P, NE * D], F16)
            for c in range(NE):
                nc.gpsimd.dma_start(out=wq_sb[:, D * c:D * (c + 1)],
                                    in_=wq[P * c:P * (c + 1), :])
                nc.gpsimd.dma_start(out=wk_sb[:, D * c:D * (c + 1)],
                                    in_=wk[P * c:P * (c + 1), :])
                nc.gpsimd.dma_start(out=wvd_sb[:, D * c:D * (c + 1)],
                                    in_=wvd[P * c:P * (c + 1), :])
            wvu_sb = cpool.tile([D, E], F16)
            nc.gpsimd.dma_start(out=wvu_sb[:], in_=wvu[:])
            mask_sb = cpool.tile([P, P], F32)
            nc.gpsimd.dma_start(out=mask_sb[:], in_=maskd[:])
            ones_sb = cpool.tile([P, 1], F16)
            nc.gpsimd.dma_start(out=ones_sb[:], in_=ones_in[:])
            ones4_sb = cpool.tile([P, 4], F16)
            for _c4 in range(4):
                nc.gpsimd.dma_start(out=ones4_sb[:, _c4:_c4 + 1], in_=ones_in[:])
            negb_sb = cpool.tile([P, 1], F32)
            nc.gpsimd.dma_start(out=negb_sb[:], in_=negb_in[:])

            def emit_xt(b):
                xt = [xtpool.tile([P, S], F16, tag="xt", name=f"xt{b}_{c}")
                      for c in range(NE)]
                # batch 0: column-quarter DMAs so the sc=0 projection group
                # unblocks after 1/4 of the data (subtile deps); later
                # batches prefetch far ahead, so fewer/larger transfers win
                nq = 4 if b == 0 else 1
                for q in range(nq):
                    ql = slice((S // nq) * q, (S // nq) * (q + 1))
                    for c in range(NE):
                        nc.sync.dma_start(out=xt[c][:, ql],
                                          in_=xT[b, P * c:P * (c + 1), ql])
                return xt

            def proj_steps(b, xt):
                """Allocate qt/kt/vd tiles for batch b; return emission thunks
                (one per PSUM group) so projection work can be interleaved
                into the previous batch's attention blocks."""
                qt_sb = qkpool.tile([D, S], F16, tag="qt", name=f"qt{b}")
                kt_sb = qkpool.tile([D, S], F16, tag="kt", name=f"kt{b}")
                vdq = [vdpool.tile([P, 4 * (D + 1)], F16, tag="vd",
                                   name=f"vdq{b}_{q}") for q in range(4)]
                vd = [vdq[t // 4][:, (D + 1) * (t % 4):(D + 1) * (t % 4 + 1)]
                      for t in range(NT)]
                steps = []

                def qk_step(sc, wsb, dst, on_dve):
                    def f():
                        sl = slice(512 * sc, 512 * (sc + 1))
                        pq = pspool.tile([D, 512], F32, tag="mm",
                                         name=f"pj{b}_{sc}_{id(wsb)}")
                        for c in range(NE):
                            nc.tensor.matmul(pq[:], wsb[:, D * c:D * (c + 1)],
                                             xt[c][:, sl],
                                             start=(c == 0), stop=(c == NE - 1))
                        if on_dve:
                            nc.vector.tensor_copy(dst[:, sl], pq[:])
                        else:
                            nc.scalar.copy(dst[:, sl], pq[:])
                    return f

                def vd_step(q):
                    def f():
                        pv = pspool.tile([P, 4 * D], F32, tag="mm",
                                         name=f"pv{b}_{q}")
                        for i in range(4):
                            t = 4 * q + i
                            for c in range(NE):
                                nc.tensor.matmul(
                                    pv[:, D * i:D * (i + 1)],
                                    xt[c][:, P * t:P * (t + 1)],
                                    wvd_sb[:, D * c:D * (c + 1)],
                                    start=(c == 0), stop=(c == NE - 1),
                                    skip_group_check=True)
                        # one strided copy: pv [128,(i d)] -> vdq [128,(i d+1)]
                        nc.vector.tensor_copy(
                            vdq[q][:, :].rearrange("p (i d) -> p i d",
                                                   d=D + 1)[:, :, 0:D],
                            pv[:, :].rearrange("p (i d) -> p i d", d=D))
                        nc.vector.tensor_copy(
                            vdq[q][:, :].rearrange("p (i d) -> p i d",
                                                   d=D + 1)[:, :, D:D + 1],
                            ones4_sb[:, :].rearrange("p (i o) -> p i o", o=1))
                    return f

                for sc in range(NBLK):
                    steps.append(qk_step(sc, wq_sb, qt_sb, False))
                    steps.append(qk_step(sc, wk_sb, kt_sb, False))
                for q in range(4):
                    steps.append(vd_step(q))
                return qt_sb, kt_sb, vd, steps

            def attention_block(b, p, qt_sb, kt_sb, vd):
                jmax = 4 * p + 3
                pod = podpool.tile([D + 1, 512], F32, tag="od",
                                   name=f"pod{b}_{p}")
                for j in range(jmax + 1):
                    w0 = P * (j - 4 * p) if j >= 4 * p else 0
                    psc = pspool.tile([P, 512], F32, tag="mm",
                                      name=f"sc{b}_{p}_{j}")
                    et = epool.tile([P, 512], F16, tag="e",
                                    name=f"et{b}_{p}_{j}")
                    nc.tensor.matmul(
                        psc[:, w0:512],
                        kt_sb[:, P * j:P * (j + 1)],
                        qt_sb[:, 512 * p + w0:512 * (p + 1)],
                        start=True, stop=True)
                    if j >= 4 * p:
                        nc.vector.tensor_add(
                            psc[:, w0:w0 + P],
                            psc[:, w0:w0 + P], mask_sb[:])
                    nc.scalar.activation(et[:, w0:512], psc[:, w0:512],
                                         Exp, scale=0.125, bias=negb_sb[:])
                    nc.tensor.matmul(pod[:, w0:512], vd[j][:],
                                     et[:, w0:512],
                                     start=(j == 0), stop=(j == jmax),
                                     skip_group_check=True)

                od_sb = odpool.tile([D + 1, 512], F16, tag="ods",
                                    name=f"ods{b}_{p}")
                nc.vector.tensor_copy(od_sb[:], pod[:])
                lrow0 = rcpool.tile([1, 512], F16, tag="lr", name=f"lr{b}_{p}")
                nc.sync.dma_start(out=lrow0[:], in_=od_sb[D:D + 1, :])
                rrow = rcpool.tile([1, 512], F16, tag="rr", name=f"rr{b}_{p}")
                with nc.allow_low_precision(reason="1/l fp16 transpose trick"):
                    nc.vector.reciprocal(rrow[:], lrow0[:])
                prec = pfpool.tile([P, 4], F32, tag="f", name=f"prec{b}_{p}")
                for c in range(4):
                    nc.tensor.matmul(prec[:, c:c + 1],
                                     rrow[0:1, P * c:P * (c + 1)],
                                     ones_sb[0:1, :],
                                     start=True, stop=True)
                rec = rcpool.tile([P, 4], F32, tag="rc", name=f"rc{b}_{p}")
                nc.vector.tensor_copy(rec[:], prec[:])
                for k in range(4):
                    pf = pfpool.tile([P, E], F32, tag="f",
                                     name=f"pf{b}_{p}_{k}")
                    nc.tensor.matmul(pf[:], od_sb[0:D, P * k:P * (k + 1)],
                                     wvu_sb[:], start=True, stop=True)
                    osb = opool.tile([P, E], F32, tag="o",
                                     name=f"o{b}_{p}_{k}")
                    nc.vector.tensor_scalar_mul(osb[:], pf[:], rec[:, k:k + 1])
                    row = 512 * p + P * k
                    nc.sync.dma_start(out=out[b, row:row + P, :], in_=osb[:])

            # prologue: batch 0 projections up-front
            xt = emit_xt(0)
            qt_sb, kt_sb, vd, _steps = None, None, None, None
            qt_sb, kt_sb, vd, st = proj_steps(0, xt)
            for f in st:
                f()
            for b in range(B):
                nxt = None
                if b + 1 < B:
                    xt_n = emit_xt(b + 1)
                    nxt = proj_steps(b + 1, xt_n)
                for p in range(NBLK):
                    attention_block(b, p, qt_sb, kt_sb, vd)
                    if nxt is not None:
                        for f in nxt[3][6 * p:6 * (p + 1)]:
                            f()
                if nxt is not None:
                    qt_sb, kt_sb, vd = nxt[0], nxt[1], nxt[2]

    nc.compile()
    return nc


def _masks():
    """mask variant k, rows [128k,128k+128): cols < 128k -> -1e6,
    diagonal sub-tile triangular (keep t<=s), cols > diag -> 0."""
    tt, ss = np.meshgrid(np.arange(P), np.arange(P), indexing="ij")
    return np.where(tt <= ss, 0.0, -1.0e6).astype(np.float32)


def kernel(x, Wq, Wk, Wvd, Wvu, Wo):
    global LAST_RESULT
    if "nc" not in _prog_cache:
        _prog_cache["nc"] = _build_program()
    nc = _prog_cache["nc"]

    xT = np.ascontiguousarray(np.asarray(x).transpose(0, 2, 1)).astype(np.float16)
    maskd = _masks()
    ones_in = np.ones((P, 1), np.float16)

    in_maps = []
    for h in range(H):
        in_maps.append({
            "xT": xT,
            "wq": np.ascontiguousarray(Wq[h]).astype(np.float16),
            "wk": np.ascontiguousarray(Wk[h]).astype(np.float16),
            "wvd": np.ascontiguousarray(Wvd[h]).astype(np.float16),
            "wvu": np.ascontiguousarray(Wvu[h]).astype(np.float16),
            "maskd": maskd,
            "ones_in": ones_in,
            "negb_in": np.full((P, 1), -EXPB, np.float32),
        })

    res = run_bass_kernel_spmd(nc, in_maps, list(range(H)))
    LAST_RESULT = res

    out = np.zeros((B, S, E), np.float32)
    wo = np.asarray(Wo, np.float32).reshape(H)
    for h in range(H):
        out += wo[h] * res.results[h]["out"].astype(np.float32)
    return out


# revision 41
# speedup vs baseline: 1.0285x; 1.0114x over previous
"""Trainium2 Bass kernel for nn_MultiHeadAttention_34651796144477.

Head-parallel sharding: core h computes the full (normalized) attention
output for head h over all 4 batches; the host applies the Wo-weighted
combine across the 8 per-head partials (the "all-reduce over heads").

Low-rank restructure (V = x @ Wvd @ Wvu has rank D=64):
  QT[d,s], KT[d,s]  : projections, d on partitions
  Vd = x @ Wvd      : [t, 64] tiles (t on partitions)
  scoreT[t,s] = K Q^T, causal-masked in PSUM, exp via ACT (scale=1/8,
                bias=-EXPB; no max-subtraction — scores are bounded)
  odT[d', s] = sum_t [Vd | 1][t, d'] exp[t, s]   (d'=65: row 64 is the
                softmax denominator l[s], accumulated in the same matmul)
  out[s, e] = (odT[0:64, s].T @ Wvu) * (e^? / l[s])  — per-partition scale

All matmul operands are fp16 (PSUM fp32); upper-triangle tiles are skipped
and diagonal-region tiles are masked with -1e6 before exp.
"""

import os
import sys

try:
    import concourse  # noqa: F401  (resolves via the axon site paths)
except ImportError:
    for _p in ("/opt/trn_rl_repo",):
        if _p not in sys.path and os.path.isdir(_p):
            sys.path.append(_p)

import numpy as np

import concourse.bacc as bacc
import concourse.mybir as mybir
from concourse.tile import TileContext
from concourse.bass_utils import run_bass_kernel_spmd

B, S, E, D, H = 4, 2048, 512, 64, 8
P = 128                 # partition tile
NT = S // P             # 16 key tiles
NBLK = 4                # s blocks of 512 (4 row tiles each)
NE = E // P             # 4 e-chunks
EXPB = 3.0              # constant bias inside exp(): keeps fp16 attn tiles
                        # and the odT accumulator in range; cancels in out/l
F32 = mybir.dt.float32
F16 = mybir.dt.float16

LAST_RESULT = None      # test harness reads exec_time_ns off this

_prog_cache = {}


def _build_program():
    nc = bacc.Bacc(None, target_bir_lowering=False)

    xT = nc.declare_dram_parameter("xT", [B, E, S], F16, isOutput=False)
    wq = nc.declare_dram_parameter("wq", [E, D], F16, isOutput=False)
    wk = nc.declare_dram_parameter("wk", [E, D], F16, isOutput=False)
    wvd = nc.declare_dram_parameter("wvd", [E, D], F16, isOutput=False)
    wvu = nc.declare_dram_parameter("wvu", [D, E], F16, isOutput=False)
    # 4 mask variants [128, 512]: variant k masks columns < 128k fully and
    # the [128k, 128k+128) sub-tile triangularly (keep t<=s)
    maskd = nc.declare_dram_parameter("maskd", [P, P], F32, isOutput=False)
    ones_in = nc.declare_dram_parameter("ones_in", [P, 1], F16, isOutput=False)
    negb_in = nc.declare_dram_parameter("negb_in", [P, 1], F32, isOutput=False)
    out = nc.declare_dram_parameter("out", [B, S, E], F32, isOutput=True)

    Exp = mybir.ActivationFunctionType.Exp

    with TileContext(nc) as tc:
        with tc.tile_pool(name="const", bufs=1) as cpool, \
             tc.tile_pool(name="xt", bufs=8) as xtpool, \
             tc.tile_pool(name="qk", bufs=4) as qkpool, \
             tc.tile_pool(name="vd", bufs=9) as vdpool, \
             tc.tile_pool(name="et", bufs=20) as epool, \
             tc.tile_pool(name="od", bufs=4) as odpool, \
             tc.tile_pool(name="ob", bufs=8) as opool, \
             tc.tile_pool(name="rc", bufs=8) as rcpool, \
             tc.tile_pool(name="ps", bufs=6, space="PSUM") as pspool, \
             tc.tile_pool(name="pod", bufs=1, space="PSUM") as podpool, \
             tc.tile_pool(name="pf", bufs=1, space="PSUM") as pfpool:

            # ---- constants / weights ----
            wq_sb = cpool.tile([P, NE * D], F16)    # chunk c: e in [128c,128c+128)
            wk_sb = cpool.tile([P, NE * D], F16)
            wvd_sb = cpool.tile([P, NE * D], F16)
            for c in range(NE):
                nc.gpsimd.dma_start(out=wq_sb[:, D * c:D * (c + 1)],
                                    in_=wq[P * c:P * (c + 1), :])
                nc.gpsimd.dma_start(out=wk_sb[:, D * c:D * (c + 1)],
                                    in_=wk[P * c:P * (c + 1), :])
                nc.gpsimd.dma_start(out=wvd_sb[:, D * c:D * (c + 1)],
                                    in_=wvd[P * c:P * (c + 1), :])
            wvu_sb = cpool.tile([D, E], F16)
            nc.gpsimd.dma_start(out=wvu_sb[:], in_=wvu[:])
            mask_sb = cpool.tile([P, P], F32)
            nc.gpsimd.dma_start(out=mask_sb[:], in_=maskd[:])
            ones_sb = cpool.tile([P, 1], F16)
            nc.gpsimd.dma_start(out=ones_sb[:], in_=ones_in[:])
            ones4_sb = cpool.tile([P, 4], F16)
            for _c4 in range(4):
                nc.gpsimd.dma_start(out=ones4_sb[:, _c4:_c4 + 1], in_=ones_in[:])
            negb_sb = cpool.tile([P, 1], F32)
            nc.gpsimd.dma_start(out=negb_sb[:], in_=negb_in[:])

            def emit_xt(b):
                xt = [xtpool.tile([P, S], F16, tag="xt", name=f"xt{b}_{c}")
                      for c in range(NE)]
                # batch 0: column-quarter DMAs so the sc=0 projection group
                # unblocks after 1/4 of the data (subtile deps); later
                # batches prefetch far ahead, so fewer/larger transfers win
                nq = 4 if b == 0 else 1
                for q in range(nq):
                    ql = slice((S // nq) * q, (S // nq) * (q + 1))
                    for c in range(NE):
                        nc.sync.dma_start(out=xt[c][:, ql],
                                          in_=xT[b, P * c:P * (c + 1), ql])
                return xt

            def proj_steps(b, xt):
                """Allocate qt/kt/vd tiles for batch b; return emission thunks
                (one per PSUM group) so projection work can be interleaved
                into the previous batch's attention blocks."""
                qt_sb = qkpool.tile([D, S], F16, tag="qt", name=f"qt{b}")
                kt_sb = qkpool.tile([D, S], F16, tag="kt", name=f"kt{b}")
                vdq = [vdpool.tile([P, 4 * (D + 1)], F16, tag="vd",
                                   name=f"vdq{b}_{q}") for q in range(4)]
                vd = [vdq[t // 4][:, (D + 1) * (t % 4):(D + 1) * (t % 4 + 1)]
                      for t in range(NT)]
                steps = []

                def qk_step(sc, wsb, dst, on_dve):
                    def f():
                        sl = slice(512 * sc, 512 * (sc + 1))
                        pq = pspool.tile([D, 512], F32, tag="mm",
                                         name=f"pj{b}_{sc}_{id(wsb)}")
                        for c in range(NE):
                            nc.tensor.matmul(pq[:], wsb[:, D * c:D * (c + 1)],
                                             xt[c][:, sl],
                                             start=(c == 0), stop=(c == NE - 1))
                        if on_dve:
                            nc.vector.tensor_copy(dst[:, sl], pq[:])
                        else:
                            nc.scalar.copy(dst[:, sl], pq[:])
                    return f

                def vd_step(q):
                    def f():
                        pv = pspool.tile([P, 4 * D], F32, tag="mm",
                                         name=f"pv{b}_{q}")
                        for i in range(4):
                            t = 4 * q + i
                            for c in range(NE):
                                nc.tensor.matmul(
                                    pv[:, D * i:D * (i + 1)],
                                    xt[c][:, P * t:P * (t + 1)],
                                    wvd_sb[:, D * c:D * (c + 1)],
                                    start=(c == 0), stop=(c == NE - 1),
                                    skip_group_check=True)
                        # one strided copy: pv [128,(i d)] -> vdq [128,(i d+1)]
                        nc.vector.tensor_copy(
                            vdq[q][:, :].rearrange("p (i d) -> p i d",
                                                   d=D + 1)[:, :, 0:D],
                            pv[:, :].rearrange("p (i d) -> p i d", d=D))
                        nc.vector.tensor_copy(
                            vdq[q][:, :].rearrange("p (i d) -> p i d",
                                                   d=D + 1)[:, :, D:D + 1],
                            ones4_sb[:, :].rearrange("p (i o) -> p i o", o=1))
                    return f

                for sc in range(NBLK):
                    steps.append(qk_step(sc, wq_sb, qt_sb, False))
                    steps.append(qk_step(sc, wk_sb, kt_sb, False))
                for q in range(4):
                    steps.append(vd_step(q))
                return qt_sb, kt_sb, vd, steps

            def attention_block(b, p, qt_sb, kt_sb, vd):
                jmax = 4 * p + 3
                pod = podpool.tile([D + 1, 512], F32, tag="od",
                                   name=f"pod{b}_{p}")
                for j in range(jmax + 1):
                    w0 = P * (j - 4 * p) if j >= 4 * p else 0
                    psc = pspool.tile([P, 512], F32, tag="mm",
                                      name=f"sc{b}_{p}_{j}")
                    et = epool.tile([P, 512], F16, tag="e",
                                    name=f"et{b}_{p}_{j}")
                    nc.tensor.matmul(
                        psc[:, w0:512],
                        kt_sb[:, P * j:P * (j + 1)],
                        qt_sb[:, 512 * p + w0:512 * (p + 1)],
                        start=True, stop=True)
                    if j >= 4 * p:
                        nc.vector.tensor_add(
                            psc[:, w0:w0 + P],
                            psc[:, w0:w0 + P], mask_sb[:])
                    nc.scalar.activation(et[:, w0:512], psc[:, w0:512],
                                         Exp, scale=0.125, bias=negb_sb[:])
                    nc.tensor.matmul(pod[:, w0:512], vd[j][:],
                                     et[:, w0:512],
                                     start=(j == 0), stop=(j == jmax),
                                     skip_group_check=True)

                od_sb = odpool.tile([D + 1, 512], F16, tag="ods",
                                    name=f"ods{b}_{p}")
                nc.vector.tensor_copy(od_sb[:], pod[:])
                lrow0 = rcpool.tile([1, 512], F16, tag="lr", name=f"lr{b}_{p}")
                nc.sync.dma_start(out=lrow0[:], in_=od_sb[D:D + 1, :])
                rrow = rcpool.tile([1, 512], F16, tag="rr", name=f"rr{b}_{p}")
                with nc.allow_low_precision(reason="1/l fp16 transpose trick"):
                    nc.vector.reciprocal(rrow[:], lrow0[:])
                prec = pfpool.tile([P, 4], F32, tag="f", name=f"prec{b}_{p}")
                for c in range(4):
                    nc.tensor.matmul(prec[:, c:c + 1],
                                     rrow[0:1, P * c:P * (c + 1)],
                                     ones_sb[0:1, :],
                                     start=True, stop=True)
                rec = rcpool.tile([P, 4], F32, tag="rc", name=f"rc{b}_{p}")
                nc.vector.tensor_copy(rec[:], prec[:])
                for k in range(4):
                    pf = pfpool.tile([P, E], F32, tag="f",
                                     name=f"pf{b}_{p}_{k}")
                    nc.tensor.matmul(pf[:], od_sb[0:D, P * k:P * (k + 1)],
                                     wvu_sb[:], start=True, stop=True)
                    osb = opool.tile([P, E], F32, tag="o",
                                     name=f"o{b}_{p}_{k}")
                    nc.vector.tensor_scalar_mul(osb[:], pf[:], rec[:, k:k + 1])
                    row = 512 * p + P * k
                    nc.sync.dma_start(out=out[b, row:row + P, :], in_=osb[:])

            # prologue: batch 0 projections up-front
            xt = emit_xt(0)
            qt_sb, kt_sb, vd, _steps = None, None, None, None
            qt_sb, kt_sb, vd, st = proj_steps(0, xt)
            for f in st:
                f()
            for b in range(B):
                nxt = None
                if b + 1 < B:
                    xt_n = emit_xt(b + 1)
                    nxt = proj_steps(b + 1, xt_n)
                for p in range(NBLK):
                    attention_block(b, p, qt_sb, kt_sb, vd)
                    if nxt is not None:
                        for f in nxt[3][6 * p:6 * (p + 1)]:
                            f()
                if nxt is not None:
                    qt_sb, kt_sb, vd = nxt[0], nxt[1], nxt[2]

    nc.compile()
    return nc


def _masks():
    """mask variant k, rows [128k,128k+128): cols < 128k -> -1e6,
    diagonal sub-tile triangular (keep t<=s), cols > diag -> 0."""
    tt, ss = np.meshgrid(np.arange(P), np.arange(P), indexing="ij")
    return np.where(tt <= ss, 0.0, -1.0e6).astype(np.float32)


def kernel(x, Wq, Wk, Wvd, Wvu, Wo):
    global LAST_RESULT
    if "nc" not in _prog_cache:
        _prog_cache["nc"] = _build_program()
    nc = _prog_cache["nc"]

    xT = np.ascontiguousarray(np.asarray(x).transpose(0, 2, 1)).astype(np.float16)
    maskd = _masks()
    ones_in = np.ones((P, 1), np.float16)

    in_maps = []
    for h in range(H):
        in_maps.append({
            "xT": xT,
            "wq": np.ascontiguousarray(Wq[h]).astype(np.float16),
            "wk": np.ascontiguousarray(Wk[h]).astype(np.float16),
            "wvd": np.ascontiguousarray(Wvd[h]).astype(np.float16),
            "wvu": np.ascontiguousarray(Wvu[h]).astype(np.float16),
            "maskd": maskd,
            "ones_in": ones_in,
            "negb_in": np.full((P, 1), -EXPB, np.float32),
        })

    res = run_bass_kernel_spmd(nc, in_maps, list(range(H)))
    LAST_RESULT = res

    out = np.zeros((B, S, E), np.float32)
    wo = np.asarray(Wo, np.float32).reshape(H)
    for h in range(H):
        out += wo[h] * res.results[h]["out"].astype(np.float32)
    return out


# revision 48
# speedup vs baseline: 1.0395x; 1.0107x over previous
"""Trainium2 Bass kernel for nn_MultiHeadAttention_34651796144477.

Head-parallel sharding: core h computes the full (normalized) attention
output for head h over all 4 batches; the host applies the Wo-weighted
combine across the 8 per-head partials (the "all-reduce over heads").

Low-rank restructure (V = x @ Wvd @ Wvu has rank D=64):
  QT[d,s], KT[d,s]  : projections, d on partitions
  Vd = x @ Wvd      : [t, 64] tiles (t on partitions)
  scoreT[t,s] = K Q^T, causal-masked in PSUM, exp via ACT (scale=1/8,
                bias=-EXPB; no max-subtraction — scores are bounded)
  odT[d', s] = sum_t [Vd | 1][t, d'] exp[t, s]   (d'=65: row 64 is the
                softmax denominator l[s], accumulated in the same matmul)
  out[s, e] = (odT[0:64, s].T @ Wvu) * (e^? / l[s])  — per-partition scale

All matmul operands are fp16 (PSUM fp32); upper-triangle tiles are skipped
and diagonal-region tiles are masked with -1e6 before exp.
"""

import os
import sys

try:
    import concourse  # noqa: F401  (resolves via the axon site paths)
except ImportError:
    for _p in ("/opt/trn_rl_repo",):
        if _p not in sys.path and os.path.isdir(_p):
            sys.path.append(_p)

import numpy as np

import concourse.bacc as bacc
import concourse.mybir as mybir
from concourse.tile import TileContext
from concourse.bass_utils import run_bass_kernel_spmd

B, S, E, D, H = 4, 2048, 512, 64, 8
P = 128                 # partition tile
NT = S // P             # 16 key tiles
NBLK = 4                # s blocks of 512 (4 row tiles each)
NE = E // P             # 4 e-chunks
EXPB = 3.0              # constant bias inside exp(): keeps fp16 attn tiles
                        # and the odT accumulator in range; cancels in out/l
F32 = mybir.dt.float32
F16 = mybir.dt.float16

LAST_RESULT = None      # test harness reads exec_time_ns off this

_prog_cache = {}


def _build_program():
    nc = bacc.Bacc(None, target_bir_lowering=False)

    xT = nc.declare_dram_parameter("xT", [B, E, S], F16, isOutput=False)
    wq = nc.declare_dram_parameter("wq", [E, D], F16, isOutput=False)
    wk = nc.declare_dram_parameter("wk", [E, D], F16, isOutput=False)
    wvd = nc.declare_dram_parameter("wvd", [E, D], F16, isOutput=False)
    wvu = nc.declare_dram_parameter("wvu", [D, E], F16, isOutput=False)
    # 4 mask variants [128, 512]: variant k masks columns < 128k fully and
    # the [128k, 128k+128) sub-tile triangularly (keep t<=s)
    maskd = nc.declare_dram_parameter("maskd", [P, P], F32, isOutput=False)
    ones_in = nc.declare_dram_parameter("ones_in", [P, 1], F16, isOutput=False)
    negb_in = nc.declare_dram_parameter("negb_in", [P, 1], F32, isOutput=False)
    out = nc.declare_dram_parameter("out", [B, S, E], F32, isOutput=True)

    Exp = mybir.ActivationFunctionType.Exp

    with TileContext(nc) as tc:
        with tc.tile_pool(name="const", bufs=1) as cpool, \
             tc.tile_pool(name="xt", bufs=8) as xtpool, \
             tc.tile_pool(name="qk", bufs=4) as qkpool, \
             tc.tile_pool(name="vd", bufs=9) as vdpool, \
             tc.tile_pool(name="et", bufs=20) as epool, \
             tc.tile_pool(name="od", bufs=4) as odpool, \
             tc.tile_pool(name="ob", bufs=8) as opool, \
             tc.tile_pool(name="rc", bufs=8) as rcpool, \
             tc.tile_pool(name="ps", bufs=6, space="PSUM") as pspool, \
             tc.tile_pool(name="pod", bufs=1, space="PSUM") as podpool, \
             tc.tile_pool(name="pf", bufs=1, space="PSUM") as pfpool:

            # ---- constants / weights ----
            wq_sb = cpool.tile([P, NE * D], F16)    # chunk c: e in [128c,128c+128)
            wk_sb = cpool.tile([P, NE * D], F16)
            wvd_sb = cpool.tile([P, NE * D], F16)
            for c in range(NE):
                nc.gpsimd.dma_start(out=wq_sb[:, D * c:D * (c + 1)],
                                    in_=wq[P * c:P * (c + 1), :])
                nc.gpsimd.dma_start(out=wk_sb[:, D * c:D * (c + 1)],
                                    in_=wk[P * c:P * (c + 1), :])
                nc.gpsimd.dma_start(out=wvd_sb[:, D * c:D * (c + 1)],
                                    in_=wvd[P * c:P * (c + 1), :])
            wvu_sb = cpool.tile([D, E], F16)
            nc.gpsimd.dma_start(out=wvu_sb[:], in_=wvu[:])
            mask_sb = cpool.tile([P, P], F32)
            nc.gpsimd.dma_start(out=mask_sb[:], in_=maskd[:])
            ones_sb = cpool.tile([P, 1], F16)
            nc.gpsimd.dma_start(out=ones_sb[:], in_=ones_in[:])
            ones4_sb = cpool.tile([P, 4], F16)
            for _c4 in range(4):
                nc.gpsimd.dma_start(out=ones4_sb[:, _c4:_c4 + 1], in_=ones_in[:])
            negb_sb = cpool.tile([P, 1], F32)
            nc.gpsimd.dma_start(out=negb_sb[:], in_=negb_in[:])

            def emit_xt(b):
                xt = [xtpool.tile([P, S], F16, tag="xt", name=f"xt{b}_{c}")
                      for c in range(NE)]
                # batch 0: column-quarter DMAs so the sc=0 projection group
                # unblocks after 1/4 of the data (subtile deps); later
                # batches prefetch far ahead, so fewer/larger transfers win
                nq = 4 if b == 0 else 1
                for q in range(nq):
                    ql = slice((S // nq) * q, (S // nq) * (q + 1))
                    for c in range(NE):
                        nc.sync.dma_start(out=xt[c][:, ql],
                                          in_=xT[b, P * c:P * (c + 1), ql])
                return xt

            def proj_steps(b, xt):
                """Allocate qt/kt/vd tiles for batch b; return emission thunks
                (one per PSUM group) so projection work can be interleaved
                into the previous batch's attention blocks."""
                qt_sb = qkpool.tile([D, S], F16, tag="qt", name=f"qt{b}")
                kt_sb = qkpool.tile([D, S], F16, tag="kt", name=f"kt{b}")
                vdq = [vdpool.tile([P, 4 * (D + 1)], F16, tag="vd",
                                   name=f"vdq{b}_{q}") for q in range(4)]
                vd = [vdq[t // 4][:, (D + 1) * (t % 4):(D + 1) * (t % 4 + 1)]
                      for t in range(NT)]
                steps = []

                def qk_step(sc, wsb, dst, on_dve):
                    def f():
                        sl = slice(512 * sc, 512 * (sc + 1))
                        pq = pspool.tile([D, 512], F32, tag="mm",
                                         name=f"pj{b}_{sc}_{id(wsb)}")
                        for c in range(NE):
                            nc.tensor.matmul(pq[:], wsb[:, D * c:D * (c + 1)],
                                             xt[c][:, sl],
                                             start=(c == 0), stop=(c == NE - 1))
                        if on_dve:
                            nc.vector.tensor_copy(dst[:, sl], pq[:])
                        else:
                            nc.scalar.copy(dst[:, sl], pq[:])
                    return f

                def vd_step(q):
                    def f():
                        pv = pspool.tile([P, 4 * D], F32, tag="mm",
                                         name=f"pv{b}_{q}")
                        for i in range(4):
                            t = 4 * q + i
                            for c in range(NE):
                                nc.tensor.matmul(
                                    pv[:, D * i:D * (i + 1)],
                                    xt[c][:, P * t:P * (t + 1)],
                                    wvd_sb[:, D * c:D * (c + 1)],
                                    start=(c == 0), stop=(c == NE - 1),
                                    skip_group_check=True)
                        # one strided copy: pv [128,(i d)] -> vdq [128,(i d+1)]
                        nc.vector.tensor_copy(
                            vdq[q][:, :].rearrange("p (i d) -> p i d",
                                                   d=D + 1)[:, :, 0:D],
                            pv[:, :].rearrange("p (i d) -> p i d", d=D))
                        nc.vector.tensor_copy(
                            vdq[q][:, :].rearrange("p (i d) -> p i d",
                                                   d=D + 1)[:, :, D:D + 1],
                            ones4_sb[:, :].rearrange("p (i o) -> p i o", o=1))
                    return f

                for sc in range(NBLK):
                    steps.append(qk_step(sc, wq_sb, qt_sb, False))
                    steps.append(qk_step(sc, wk_sb, kt_sb, False))
                for q in range(4):
                    steps.append(vd_step(q))
                return qt_sb, kt_sb, vd, steps

            def attention_block(b, p, qt_sb, kt_sb, vd):
                jmax = 4 * p + 3
                pod = podpool.tile([D + 1, 512], F32, tag="od",
                                   name=f"pod{b}_{p}")
                for j in range(jmax + 1):
                    w0 = P * (j - 4 * p) if j >= 4 * p else 0
                    psc = pspool.tile([P, 512], F32, tag="mm",
                                      name=f"sc{b}_{p}_{j}")
                    et = epool.tile([P, 512], F16, tag="e",
                                    name=f"et{b}_{p}_{j}")
                    nc.tensor.matmul(
                        psc[:, w0:512],
                        kt_sb[:, P * j:P * (j + 1)],
                        qt_sb[:, 512 * p + w0:512 * (p + 1)],
                        start=True, stop=True)
                    if j >= 4 * p:
                        nc.vector.tensor_add(
                            psc[:, w0:w0 + P],
                            psc[:, w0:w0 + P], mask_sb[:])
                    nc.scalar.activation(et[:, w0:512], psc[:, w0:512],
                                         Exp, scale=0.125, bias=negb_sb[:])
                    nc.tensor.matmul(pod[:, w0:512], vd[j][:],
                                     et[:, w0:512],
                                     start=(j == 0), stop=(j == jmax),
                                     skip_group_check=True)

                od_sb = odpool.tile([D + 1, 512], F16, tag="ods",
                                    name=f"ods{b}_{p}")
                nc.vector.tensor_copy(od_sb[:], pod[:])
                lrow0 = rcpool.tile([1, 512], F16, tag="lr", name=f"lr{b}_{p}")
                nc.sync.dma_start(out=lrow0[:], in_=od_sb[D:D + 1, :])
                rrow = rcpool.tile([1, 512], F16, tag="rr", name=f"rr{b}_{p}")
                with nc.allow_low_precision(reason="1/l fp16 transpose trick"):
                    nc.vector.reciprocal(rrow[:], lrow0[:])
                prec = pfpool.tile([P, 4], F32, tag="f", name=f"prec{b}_{p}")
                for c in range(4):
                    nc.tensor.matmul(prec[:, c:c + 1],
                                     rrow[0:1, P * c:P * (c + 1)],
                                     ones_sb[0:1, :],
                                     start=True, stop=True)
                rec = rcpool.tile([P, 4], F32, tag="rc", name=f"rc{b}_{p}")
                nc.vector.tensor_copy(rec[:], prec[:])
                for k in range(4):
                    pf = pfpool.tile([P, E], F32, tag="f",
                                     name=f"pf{b}_{p}_{k}")
                    nc.tensor.matmul(pf[:], od_sb[0:D, P * k:P * (k + 1)],
                                     wvu_sb[:], start=True, stop=True)
                    osb = opool.tile([P, E], F32, tag="o",
                                     name=f"o{b}_{p}_{k}")
                    nc.vector.tensor_scalar_mul(osb[:], pf[:], rec[:, k:k + 1])
                    row = 512 * p + P * k
                    nc.sync.dma_start(out=out[b, row:row + P, :], in_=osb[:])

            # prologue: batch 0 projections up-front
            xt = emit_xt(0)
            qt_sb, kt_sb, vd, _steps = None, None, None, None
            qt_sb, kt_sb, vd, st = proj_steps(0, xt)
            for f in st:
                f()
            for b in range(B):
                nxt = None
                if b + 1 < B:
                    xt_n = emit_xt(b + 1)
                    nxt = proj_steps(b + 1, xt_n)
                for p in range(NBLK):
                    attention_block(b, p, qt_sb, kt_sb, vd)
                    if nxt is not None:
                        for f in nxt[3][10 * p:10 * (p + 1)]:
                            f()
                if nxt is not None:
                    qt_sb, kt_sb, vd = nxt[0], nxt[1], nxt[2]

    nc.compile()
    return nc


def _masks():
    """mask variant k, rows [128k,128k+128): cols < 128k -> -1e6,
    diagonal sub-tile triangular (keep t<=s), cols > diag -> 0."""
    tt, ss = np.meshgrid(np.arange(P), np.arange(P), indexing="ij")
    return np.where(tt <= ss, 0.0, -1.0e6).astype(np.float32)


def kernel(x, Wq, Wk, Wvd, Wvu, Wo):
    global LAST_RESULT
    if "nc" not in _prog_cache:
        _prog_cache["nc"] = _build_program()
    nc = _prog_cache["nc"]

    xT = np.ascontiguousarray(np.asarray(x).transpose(0, 2, 1)).astype(np.float16)
    maskd = _masks()
    ones_in = np.ones((P, 1), np.float16)

    in_maps = []
    for h in range(H):
        in_maps.append({
            "xT": xT,
            "wq": np.ascontiguousarray(Wq[h]).astype(np.float16),
            "wk": np.ascontiguousarray(Wk[h]).astype(np.float16),
            "wvd": np.ascontiguousarray(Wvd[h]).astype(np.float16),
            "wvu": np.ascontiguousarray(Wvu[h]).astype(np.float16),
            "maskd": maskd,
            "ones_in": ones_in,
            "negb_in": np.full((P, 1), -EXPB, np.float32),
        })

    res = run_bass_kernel_spmd(nc, in_maps, list(range(H)))
    LAST_RESULT = res

    out = np.zeros((B, S, E), np.float32)
    wo = np.asarray(Wo, np.float32).reshape(H)
    for h in range(H):
        out += wo[h] * res.results[h]["out"].astype(np.float32)
    return out


# revision 54
# speedup vs baseline: 1.0775x; 1.0365x over previous
"""Trainium2 Bass kernel for nn_MultiHeadAttention_34651796144477.

Head-parallel sharding: core h computes the full (normalized) attention
output for head h over all 4 batches; the host applies the Wo-weighted
combine across the 8 per-head partials (the "all-reduce over heads").

Low-rank restructure (V = x @ Wvd @ Wvu has rank D=64):
  QT[d,s], KT[d,s]  : projections, d on partitions
  Vd = x @ Wvd      : [t, 64] tiles (t on partitions)
  scoreT[t,s] = K Q^T, causal-masked in PSUM, exp via ACT (scale=1/8,
                bias=-EXPB; no max-subtraction — scores are bounded)
  odT[d', s] = sum_t [Vd | 1][t, d'] exp[t, s]   (d'=65: row 64 is the
                softmax denominator l[s], accumulated in the same matmul)
  out[s, e] = (odT[0:64, s].T @ Wvu) * (e^? / l[s])  — per-partition scale

All matmul operands are fp16 (PSUM fp32); upper-triangle tiles are skipped
and diagonal-region tiles are masked with -1e6 before exp.
"""

import os
import sys

try:
    import concourse  # noqa: F401  (resolves via the axon site paths)
except ImportError:
    for _p in ("/opt/trn_rl_repo",):
        if _p not in sys.path and os.path.isdir(_p):
            sys.path.append(_p)

import numpy as np

import concourse.bacc as bacc
import concourse.mybir as mybir
from concourse.tile import TileContext
from concourse.bass_utils import run_bass_kernel_spmd

B, S, E, D, H = 4, 2048, 512, 64, 8
P = 128                 # partition tile
NT = S // P             # 16 key tiles
NBLK = 4                # s blocks of 512 (4 row tiles each)
NE = E // P             # 4 e-chunks
EXPB = 3.0              # constant bias inside exp(): keeps fp16 attn tiles
                        # and the odT accumulator in range; cancels in out/l
F32 = mybir.dt.float32
F16 = mybir.dt.float16

LAST_RESULT = None      # test harness reads exec_time_ns off this

_prog_cache = {}


def _build_program():
    nc = bacc.Bacc(None, target_bir_lowering=False)

    xT = nc.declare_dram_parameter("xT", [B, E, S], F16, isOutput=False)
    wq = nc.declare_dram_parameter("wq", [E, D], F16, isOutput=False)
    wk = nc.declare_dram_parameter("wk", [E, D], F16, isOutput=False)
    wvd = nc.declare_dram_parameter("wvd", [E, D], F16, isOutput=False)
    wvu = nc.declare_dram_parameter("wvu", [D, E], F16, isOutput=False)
    # 4 mask variants [128, 512]: variant k masks columns < 128k fully and
    # the [128k, 128k+128) sub-tile triangularly (keep t<=s)
    maskd = nc.declare_dram_parameter("maskd", [P, P], F32, isOutput=False)
    ones_in = nc.declare_dram_parameter("ones_in", [P, 1], F16, isOutput=False)
    negb_in = nc.declare_dram_parameter("negb_in", [P, 1], F32, isOutput=False)
    out = nc.declare_dram_parameter("out", [B, S, E], F32, isOutput=True)

    Exp = mybir.ActivationFunctionType.Exp

    with TileContext(nc) as tc:
        with tc.tile_pool(name="const", bufs=1) as cpool, \
             tc.tile_pool(name="xt", bufs=8) as xtpool, \
             tc.tile_pool(name="qk", bufs=4) as qkpool, \
             tc.tile_pool(name="vd", bufs=9) as vdpool, \
             tc.tile_pool(name="et", bufs=20) as epool, \
             tc.tile_pool(name="od", bufs=4) as odpool, \
             tc.tile_pool(name="ob", bufs=8) as opool, \
             tc.tile_pool(name="rc", bufs=8) as rcpool, \
             tc.tile_pool(name="ps", bufs=6, space="PSUM") as pspool, \
             tc.tile_pool(name="pod", bufs=1, space="PSUM") as podpool, \
             tc.tile_pool(name="pf", bufs=1, space="PSUM") as pfpool:

            # ---- constants / weights ----
            wq_sb = cpool.tile([P, NE * D], F16)    # chunk c: e in [128c,128c+128)
            wk_sb = cpool.tile([P, NE * D], F16)
            wvd_sb = cpool.tile([P, NE * D], F16)
            for c in range(NE):
                nc.gpsimd.dma_start(out=wq_sb[:, D * c:D * (c + 1)],
                                    in_=wq[P * c:P * (c + 1), :])
                nc.gpsimd.dma_start(out=wk_sb[:, D * c:D * (c + 1)],
                                    in_=wk[P * c:P * (c + 1), :])
                nc.gpsimd.dma_start(out=wvd_sb[:, D * c:D * (c + 1)],
                                    in_=wvd[P * c:P * (c + 1), :])
            wvu_sb = cpool.tile([D, E], F16)
            nc.gpsimd.dma_start(out=wvu_sb[:], in_=wvu[:])
            mask_sb = cpool.tile([P, P], F32)
            nc.gpsimd.dma_start(out=mask_sb[:], in_=maskd[:])
            ones_sb = cpool.tile([P, 1], F16)
            nc.gpsimd.dma_start(out=ones_sb[:], in_=ones_in[:])
            ones4_sb = cpool.tile([P, 4], F16)
            for _c4 in range(4):
                nc.gpsimd.dma_start(out=ones4_sb[:, _c4:_c4 + 1], in_=ones_in[:])
            negb_sb = cpool.tile([P, 1], F32)
            nc.gpsimd.dma_start(out=negb_sb[:], in_=negb_in[:])

            def emit_xt(b):
                xt = [xtpool.tile([P, S], F16, tag="xt", name=f"xt{b}_{c}")
                      for c in range(NE)]
                # batch 0: column-quarter DMAs so the sc=0 projection group
                # unblocks after 1/4 of the data (subtile deps); later
                # batches prefetch far ahead, so fewer/larger transfers win
                nq = 4 if b == 0 else 1
                for q in range(nq):
                    ql = slice((S // nq) * q, (S // nq) * (q + 1))
                    for c in range(NE):
                        nc.sync.dma_start(out=xt[c][:, ql],
                                          in_=xT[b, P * c:P * (c + 1), ql])
                return xt

            def proj_steps(b, xt):
                """Allocate qt/kt/vd tiles for batch b; return emission thunks
                (one per PSUM group) so projection work can be interleaved
                into the previous batch's attention blocks."""
                qt_sb = qkpool.tile([D, S], F16, tag="qt", name=f"qt{b}")
                kt_sb = qkpool.tile([D, S], F16, tag="kt", name=f"kt{b}")
                vdq = [vdpool.tile([P, 4 * (D + 1)], F16, tag="vd",
                                   name=f"vdq{b}_{q}") for q in range(4)]
                vd = [vdq[t // 4][:, (D + 1) * (t % 4):(D + 1) * (t % 4 + 1)]
                      for t in range(NT)]
                steps = []

                def qk_step(sc, wsb, dst, on_dve):
                    def f():
                        sl = slice(512 * sc, 512 * (sc + 1))
                        pq = pspool.tile([D, 512], F32, tag="mm",
                                         name=f"pj{b}_{sc}_{id(wsb)}")
                        for c in range(NE):
                            nc.tensor.matmul(pq[:], wsb[:, D * c:D * (c + 1)],
                                             xt[c][:, sl],
                                             start=(c == 0), stop=(c == NE - 1))
                        if on_dve:
                            nc.vector.tensor_copy(dst[:, sl], pq[:])
                        else:
                            nc.scalar.copy(dst[:, sl], pq[:])
                    return f

                def vd_step(q):
                    def f():
                        pv = pspool.tile([P, 4 * D], F32, tag="mm",
                                         name=f"pv{b}_{q}")
                        for i in range(4):
                            t = 4 * q + i
                            for c in range(NE):
                                nc.tensor.matmul(
                                    pv[:, D * i:D * (i + 1)],
                                    xt[c][:, P * t:P * (t + 1)],
                                    wvd_sb[:, D * c:D * (c + 1)],
                                    start=(c == 0), stop=(c == NE - 1),
                                    skip_group_check=True)
                        # one strided copy: pv [128,(i d)] -> vdq [128,(i d+1)]
                        nc.vector.tensor_copy(
                            vdq[q][:, :].rearrange("p (i d) -> p i d",
                                                   d=D + 1)[:, :, 0:D],
                            pv[:, :].rearrange("p (i d) -> p i d", d=D))
                        nc.vector.tensor_copy(
                            vdq[q][:, :].rearrange("p (i d) -> p i d",
                                                   d=D + 1)[:, :, D:D + 1],
                            ones4_sb[:, :].rearrange("p (i o) -> p i o", o=1))
                    return f

                for sc in range(NBLK):
                    steps.append(qk_step(sc, wq_sb, qt_sb, False))
                    steps.append(qk_step(sc, wk_sb, kt_sb, False))
                for q in range(4):
                    steps.append(vd_step(q))
                return qt_sb, kt_sb, vd, steps

            def attention_block(b, p, qt_sb, kt_sb, vd, feeder=None):
                jmax = 4 * p + 3
                pod = podpool.tile([D + 1, 512], F32, tag="od",
                                   name=f"pod{b}_{p}")
                for j in range(jmax + 1):
                    w0 = P * (j - 4 * p) if j >= 4 * p else 0
                    psc = pspool.tile([P, 512], F32, tag="mm",
                                      name=f"sc{b}_{p}_{j}")
                    et = epool.tile([P, 512], F16, tag="e",
                                    name=f"et{b}_{p}_{j}")
                    nc.tensor.matmul(
                        psc[:, w0:512],
                        kt_sb[:, P * j:P * (j + 1)],
                        qt_sb[:, 512 * p + w0:512 * (p + 1)],
                        start=True, stop=True)
                    if j >= 4 * p:
                        nc.vector.tensor_add(
                            psc[:, w0:w0 + P],
                            psc[:, w0:w0 + P], mask_sb[:])
                    nc.scalar.activation(et[:, w0:512], psc[:, w0:512],
                                         Exp, scale=0.125, bias=negb_sb[:])
                    nc.tensor.matmul(pod[:, w0:512], vd[j][:],
                                     et[:, w0:512],
                                     start=(j == 0), stop=(j == jmax),
                                     skip_group_check=True)
                    if feeder and j % 3 == 1:
                        f = next(feeder, None)
                        if f is not None:
                            f()

                od_sb = odpool.tile([D + 1, 512], F16, tag="ods",
                                    name=f"ods{b}_{p}")
                nc.vector.tensor_copy(od_sb[:], pod[:])
                lrow0 = rcpool.tile([1, 512], F16, tag="lr", name=f"lr{b}_{p}")
                nc.sync.dma_start(out=lrow0[:], in_=od_sb[D:D + 1, :])
                rrow = rcpool.tile([1, 512], F16, tag="rr", name=f"rr{b}_{p}")
                with nc.allow_low_precision(reason="1/l fp16 transpose trick"):
                    nc.vector.reciprocal(rrow[:], lrow0[:])
                prec = pfpool.tile([P, 4], F32, tag="f", name=f"prec{b}_{p}")
                for c in range(4):
                    nc.tensor.matmul(prec[:, c:c + 1],
                                     rrow[0:1, P * c:P * (c + 1)],
                                     ones_sb[0:1, :],
                                     start=True, stop=True)
                rec = rcpool.tile([P, 4], F32, tag="rc", name=f"rc{b}_{p}")
                nc.vector.tensor_copy(rec[:], prec[:])
                for k in range(4):
                    pf = pfpool.tile([P, E], F32, tag="f",
                                     name=f"pf{b}_{p}_{k}")
                    nc.tensor.matmul(pf[:], od_sb[0:D, P * k:P * (k + 1)],
                                     wvu_sb[:], start=True, stop=True)
                    osb = opool.tile([P, E], F32, tag="o",
                                     name=f"o{b}_{p}_{k}")
                    nc.vector.tensor_scalar_mul(osb[:], pf[:], rec[:, k:k + 1])
                    row = 512 * p + P * k
                    nc.sync.dma_start(out=out[b, row:row + P, :], in_=osb[:])

            # prologue: batch 0 projections up-front
            xt = emit_xt(0)
            qt_sb, kt_sb, vd, _steps = None, None, None, None
            qt_sb, kt_sb, vd, st = proj_steps(0, xt)
            for f in st:
                f()
            for b in range(B):
                nxt = None
                if b + 1 < B:
                    xt_n = emit_xt(b + 1)
                    nxt = proj_steps(b + 1, xt_n)
                feeder = iter(nxt[3]) if nxt is not None else None
                for p in range(NBLK):
                    attention_block(b, p, qt_sb, kt_sb, vd, feeder)
                if feeder is not None:
                    for f in feeder:
                        f()
                if nxt is not None:
                    qt_sb, kt_sb, vd = nxt[0], nxt[1], nxt[2]

    nc.compile()
    return nc


def _masks():
    """mask variant k, rows [128k,128k+128): cols < 128k -> -1e6,
    diagonal sub-tile triangular (keep t<=s), cols > diag -> 0."""
    tt, ss = np.meshgrid(np.arange(P), np.arange(P), indexing="ij")
    return np.where(tt <= ss, 0.0, -1.0e6).astype(np.float32)


def kernel(x, Wq, Wk, Wvd, Wvu, Wo):
    global LAST_RESULT
    if "nc" not in _prog_cache:
        _prog_cache["nc"] = _build_program()
    nc = _prog_cache["nc"]

    xT = np.ascontiguousarray(np.asarray(x).transpose(0, 2, 1)).astype(np.float16)
    maskd = _masks()
    ones_in = np.ones((P, 1), np.float16)

    in_maps = []
    for h in range(H):
        in_maps.append({
            "xT": xT,
            "wq": np.ascontiguousarray(Wq[h]).astype(np.float16),
            "wk": np.ascontiguousarray(Wk[h]).astype(np.float16),
            "wvd": np.ascontiguousarray(Wvd[h]).astype(np.float16),
            "wvu": np.ascontiguousarray(Wvu[h]).astype(np.float16),
            "maskd": maskd,
            "ones_in": ones_in,
            "negb_in": np.full((P, 1), -EXPB, np.float32),
        })

    res = run_bass_kernel_spmd(nc, in_maps, list(range(H)))
    LAST_RESULT = res

    out = np.zeros((B, S, E), np.float32)
    wo = np.asarray(Wo, np.float32).reshape(H)
    for h in range(H):
        out += wo[h] * res.results[h]["out"].astype(np.float32)
    return out


# revision 57
# speedup vs baseline: 1.0799x; 1.0022x over previous
"""Trainium2 Bass kernel for nn_MultiHeadAttention_34651796144477.

Head-parallel sharding: core h computes the full (normalized) attention
output for head h over all 4 batches; the host applies the Wo-weighted
combine across the 8 per-head partials (the "all-reduce over heads").

Low-rank restructure (V = x @ Wvd @ Wvu has rank D=64):
  QT[d,s], KT[d,s]  : projections, d on partitions
  Vd = x @ Wvd      : [t, 64] tiles (t on partitions)
  scoreT[t,s] = K Q^T, causal-masked in PSUM, exp via ACT (scale=1/8,
                bias=-EXPB; no max-subtraction — scores are bounded)
  odT[d', s] = sum_t [Vd | 1][t, d'] exp[t, s]   (d'=65: row 64 is the
                softmax denominator l[s], accumulated in the same matmul)
  out[s, e] = (odT[0:64, s].T @ Wvu) * (e^? / l[s])  — per-partition scale

All matmul operands are fp16 (PSUM fp32); upper-triangle tiles are skipped
and diagonal-region tiles are masked with -1e6 before exp.
"""

import os
import sys

try:
    import concourse  # noqa: F401  (resolves via the axon site paths)
except ImportError:
    for _p in ("/opt/trn_rl_repo",):
        if _p not in sys.path and os.path.isdir(_p):
            sys.path.append(_p)

import numpy as np

import concourse.bacc as bacc
import concourse.mybir as mybir
from concourse.tile import TileContext
from concourse.bass_utils import run_bass_kernel_spmd

B, S, E, D, H = 4, 2048, 512, 64, 8
P = 128                 # partition tile
NT = S // P             # 16 key tiles
NBLK = 4                # s blocks of 512 (4 row tiles each)
NE = E // P             # 4 e-chunks
EXPB = 3.0              # constant bias inside exp(): keeps fp16 attn tiles
                        # and the odT accumulator in range; cancels in out/l
F32 = mybir.dt.float32
F16 = mybir.dt.float16

LAST_RESULT = None      # test harness reads exec_time_ns off this

_prog_cache = {}


def _build_program():
    nc = bacc.Bacc(None, target_bir_lowering=False)

    xT = nc.declare_dram_parameter("xT", [B, E, S], F16, isOutput=False)
    wq = nc.declare_dram_parameter("wq", [E, D], F16, isOutput=False)
    wk = nc.declare_dram_parameter("wk", [E, D], F16, isOutput=False)
    wvd = nc.declare_dram_parameter("wvd", [E, D], F16, isOutput=False)
    wvu = nc.declare_dram_parameter("wvu", [D, E], F16, isOutput=False)
    # 4 mask variants [128, 512]: variant k masks columns < 128k fully and
    # the [128k, 128k+128) sub-tile triangularly (keep t<=s)
    maskd = nc.declare_dram_parameter("maskd", [P, P], F32, isOutput=False)
    ones_in = nc.declare_dram_parameter("ones_in", [P, 1], F16, isOutput=False)
    negb_in = nc.declare_dram_parameter("negb_in", [P, 1], F32, isOutput=False)
    out = nc.declare_dram_parameter("out", [B, S, E], F32, isOutput=True)

    Exp = mybir.ActivationFunctionType.Exp

    with TileContext(nc) as tc:
        with tc.tile_pool(name="const", bufs=1) as cpool, \
             tc.tile_pool(name="xt", bufs=8) as xtpool, \
             tc.tile_pool(name="qk", bufs=4) as qkpool, \
             tc.tile_pool(name="vd", bufs=9) as vdpool, \
             tc.tile_pool(name="et", bufs=20) as epool, \
             tc.tile_pool(name="od", bufs=4) as odpool, \
             tc.tile_pool(name="ob", bufs=8) as opool, \
             tc.tile_pool(name="rc", bufs=8) as rcpool, \
             tc.tile_pool(name="ps", bufs=6, space="PSUM") as pspool, \
             tc.tile_pool(name="pod", bufs=1, space="PSUM") as podpool, \
             tc.tile_pool(name="pf", bufs=1, space="PSUM") as pfpool:

            # ---- constants / weights ----
            wq_sb = cpool.tile([P, NE * D], F16)    # chunk c: e in [128c,128c+128)
            wk_sb = cpool.tile([P, NE * D], F16)
            wvd_sb = cpool.tile([P, NE * D], F16)
            for c in range(NE):
                nc.gpsimd.dma_start(out=wq_sb[:, D * c:D * (c + 1)],
                                    in_=wq[P * c:P * (c + 1), :])
                nc.gpsimd.dma_start(out=wk_sb[:, D * c:D * (c + 1)],
                                    in_=wk[P * c:P * (c + 1), :])
                nc.gpsimd.dma_start(out=wvd_sb[:, D * c:D * (c + 1)],
                                    in_=wvd[P * c:P * (c + 1), :])
            wvu_sb = cpool.tile([D, E], F16)
            nc.gpsimd.dma_start(out=wvu_sb[:], in_=wvu[:])
            mask_sb = cpool.tile([P, P], F32)
            nc.gpsimd.dma_start(out=mask_sb[:], in_=maskd[:])
            ones_sb = cpool.tile([P, 1], F16)
            nc.gpsimd.dma_start(out=ones_sb[:], in_=ones_in[:])
            ones4_sb = cpool.tile([P, 4], F16)
            for _c4 in range(4):
                nc.gpsimd.dma_start(out=ones4_sb[:, _c4:_c4 + 1], in_=ones_in[:])
            negb_sb = cpool.tile([P, 1], F32)
            nc.gpsimd.dma_start(out=negb_sb[:], in_=negb_in[:])

            def emit_xt(b):
                xt = [xtpool.tile([P, S], F16, tag="xt", name=f"xt{b}_{c}")
                      for c in range(NE)]
                # batch 0: column-quarter DMAs so the sc=0 projection group
                # unblocks after 1/4 of the data (subtile deps); later
                # batches prefetch far ahead, so fewer/larger transfers win
                nq = 4 if b == 0 else 1
                for q in range(nq):
                    ql = slice((S // nq) * q, (S // nq) * (q + 1))
                    for c in range(NE):
                        nc.sync.dma_start(out=xt[c][:, ql],
                                          in_=xT[b, P * c:P * (c + 1), ql])
                return xt

            def proj_steps(b, xt):
                """Allocate qt/kt/vd tiles for batch b; return emission thunks
                (one per PSUM group) so projection work can be interleaved
                into the previous batch's attention blocks."""
                qt_sb = qkpool.tile([D, S], F16, tag="qt", name=f"qt{b}")
                kt_sb = qkpool.tile([D, S], F16, tag="kt", name=f"kt{b}")
                vdq = [vdpool.tile([P, 4 * (D + 1)], F16, tag="vd",
                                   name=f"vdq{b}_{q}") for q in range(4)]
                vd = [vdq[t // 4][:, (D + 1) * (t % 4):(D + 1) * (t % 4 + 1)]
                      for t in range(NT)]
                steps = []

                def qk_step(sc, wsb, dst, on_dve):
                    def f():
                        sl = slice(512 * sc, 512 * (sc + 1))
                        pq = pspool.tile([D, 512], F32, tag="mm",
                                         name=f"pj{b}_{sc}_{id(wsb)}")
                        for c in range(NE):
                            nc.tensor.matmul(pq[:], wsb[:, D * c:D * (c + 1)],
                                             xt[c][:, sl],
                                             start=(c == 0), stop=(c == NE - 1))
                        if on_dve:
                            nc.vector.tensor_copy(dst[:, sl], pq[:])
                        else:
                            nc.scalar.copy(dst[:, sl], pq[:])
                    return f

                def vd_step(q):
                    def f():
                        pv = pspool.tile([P, 4 * D], F32, tag="mm",
                                         name=f"pv{b}_{q}")
                        for i in range(4):
                            t = 4 * q + i
                            for c in range(NE):
                                nc.tensor.matmul(
                                    pv[:, D * i:D * (i + 1)],
                                    xt[c][:, P * t:P * (t + 1)],
                                    wvd_sb[:, D * c:D * (c + 1)],
                                    start=(c == 0), stop=(c == NE - 1),
                                    skip_group_check=True)
                        # one strided copy: pv [128,(i d)] -> vdq [128,(i d+1)]
                        nc.vector.tensor_copy(
                            vdq[q][:, :].rearrange("p (i d) -> p i d",
                                                   d=D + 1)[:, :, 0:D],
                            pv[:, :].rearrange("p (i d) -> p i d", d=D))
                        nc.vector.tensor_copy(
                            vdq[q][:, :].rearrange("p (i d) -> p i d",
                                                   d=D + 1)[:, :, D:D + 1],
                            ones4_sb[:, :].rearrange("p (i o) -> p i o", o=1))
                    return f

                for sc in range(NBLK):
                    steps.append(qk_step(sc, wq_sb, qt_sb, False))
                    steps.append(qk_step(sc, wk_sb, kt_sb, False))
                for q in range(4):
                    steps.append(vd_step(q))
                return qt_sb, kt_sb, vd, steps

            def attention_block(b, p, qt_sb, kt_sb, vd, feeder=None):
                jmax = 4 * p + 3
                pod = podpool.tile([D + 1, 512], F32, tag="od",
                                   name=f"pod{b}_{p}")
                for j in range(jmax + 1):
                    w0 = P * (j - 4 * p) if j >= 4 * p else 0
                    psc = pspool.tile([P, 512], F32, tag="mm",
                                      name=f"sc{b}_{p}_{j}")
                    et = epool.tile([P, 512], F16, tag="e",
                                    name=f"et{b}_{p}_{j}")
                    nc.tensor.matmul(
                        psc[:, w0:512],
                        kt_sb[:, P * j:P * (j + 1)],
                        qt_sb[:, 512 * p + w0:512 * (p + 1)],
                        start=True, stop=True)
                    if j >= 4 * p:
                        nc.vector.tensor_add(
                            psc[:, w0:w0 + P],
                            psc[:, w0:w0 + P], mask_sb[:])
                    nc.scalar.activation(et[:, w0:512], psc[:, w0:512],
                                         Exp, scale=0.125, bias=negb_sb[:])
                    nc.tensor.matmul(pod[:, w0:512], vd[j][:],
                                     et[:, w0:512],
                                     start=(j == 0), stop=(j == jmax),
                                     skip_group_check=True)
                    if feeder and j % 3 == 0:
                        f = next(feeder, None)
                        if f is not None:
                            f()

                od_sb = odpool.tile([D + 1, 512], F16, tag="ods",
                                    name=f"ods{b}_{p}")
                nc.vector.tensor_copy(od_sb[:], pod[:])
                lrow0 = rcpool.tile([1, 512], F16, tag="lr", name=f"lr{b}_{p}")
                nc.sync.dma_start(out=lrow0[:], in_=od_sb[D:D + 1, :])
                rrow = rcpool.tile([1, 512], F16, tag="rr", name=f"rr{b}_{p}")
                with nc.allow_low_precision(reason="1/l fp16 transpose trick"):
                    nc.vector.reciprocal(rrow[:], lrow0[:])
                prec = pfpool.tile([P, 4], F32, tag="f", name=f"prec{b}_{p}")
                for c in range(4):
                    nc.tensor.matmul(prec[:, c:c + 1],
                                     rrow[0:1, P * c:P * (c + 1)],
                                     ones_sb[0:1, :],
                                     start=True, stop=True)
                rec = rcpool.tile([P, 4], F32, tag="rc", name=f"rc{b}_{p}")
                nc.vector.tensor_copy(rec[:], prec[:])
                for k in range(4):
                    pf = pfpool.tile([P, E], F32, tag="f",
                                     name=f"pf{b}_{p}_{k}")
                    nc.tensor.matmul(pf[:], od_sb[0:D, P * k:P * (k + 1)],
                                     wvu_sb[:], start=True, stop=True)
                    osb = opool.tile([P, E], F32, tag="o",
                                     name=f"o{b}_{p}_{k}")
                    nc.vector.tensor_scalar_mul(osb[:], pf[:], rec[:, k:k + 1])
                    row = 512 * p + P * k
                    nc.sync.dma_start(out=out[b, row:row + P, :], in_=osb[:])

            # prologue: batch 0 projections up-front
            xt = emit_xt(0)
            qt_sb, kt_sb, vd, _steps = None, None, None, None
            qt_sb, kt_sb, vd, st = proj_steps(0, xt)
            for f in st:
                f()
            for b in range(B):
                nxt = None
                if b + 1 < B:
                    xt_n = emit_xt(b + 1)
                    nxt = proj_steps(b + 1, xt_n)
                feeder = iter(nxt[3]) if nxt is not None else None
                for p in range(NBLK):
                    attention_block(b, p, qt_sb, kt_sb, vd, feeder)
                if feeder is not None:
                    for f in feeder:
                        f()
                if nxt is not None:
                    qt_sb, kt_sb, vd = nxt[0], nxt[1], nxt[2]

    nc.compile()
    return nc


def _masks():
    """mask variant k, rows [128k,128k+128): cols < 128k -> -1e6,
    diagonal sub-tile triangular (keep t<=s), cols > diag -> 0."""
    tt, ss = np.meshgrid(np.arange(P), np.arange(P), indexing="ij")
    return np.where(tt <= ss, 0.0, -1.0e6).astype(np.float32)


def kernel(x, Wq, Wk, Wvd, Wvu, Wo):
    global LAST_RESULT
    if "nc" not in _prog_cache:
        _prog_cache["nc"] = _build_program()
    nc = _prog_cache["nc"]

    xT = np.ascontiguousarray(np.asarray(x).transpose(0, 2, 1)).astype(np.float16)
    maskd = _masks()
    ones_in = np.ones((P, 1), np.float16)

    in_maps = []
    for h in range(H):
        in_maps.append({
            "xT": xT,
            "wq": np.ascontiguousarray(Wq[h]).astype(np.float16),
            "wk": np.ascontiguousarray(Wk[h]).astype(np.float16),
            "wvd": np.ascontiguousarray(Wvd[h]).astype(np.float16),
            "wvu": np.ascontiguousarray(Wvu[h]).astype(np.float16),
            "maskd": maskd,
            "ones_in": ones_in,
            "negb_in": np.full((P, 1), -EXPB, np.float32),
        })

    res = run_bass_kernel_spmd(nc, in_maps, list(range(H)))
    LAST_RESULT = res

    out = np.zeros((B, S, E), np.float32)
    wo = np.asarray(Wo, np.float32).reshape(H)
    for h in range(H):
        out += wo[h] * res.results[h]["out"].astype(np.float32)
    return out
